# revision 9
# baseline (speedup 1.0000x reference)
"""Phase B: fp32r split-precision kernel (~22-bit effective, PE at full rate).

Every logical fp32 matmul x@W is computed as 3 fp32r matmuls
  x_hi@W_hi + x_lo@W_hi + x_hi@W_lo        (error ~2^-22, PSUM fp32)
with hi = rne11(x), lo = rne11(x - hi)  (fp32r = fp32 container, 11-bit
mantissa, full PE rate vs 1/4 for fp32).
Weights are split on the host. Activations h are split at runtime (DVE).
Backward layer-2 uses the exactness trick: g2 = w*m2 with m2 in {1, 0.5}
=> g2 @ A2^T == m2 @ B2 with B2 = diag(w)A2^T host-split, and m2 is
EXACTLY representable in fp32r, so only 2 matmuls (B2_hi, B2_lo).
Same for the layer-2 grad contribution via C2T = diag(w)W2^T.
"""

import numpy as np

import concourse.bass as bass
import concourse.mybir as mybir
import concourse.tile as tile
from concourse import bacc
from concourse.bass_utils import run_bass_kernel_spmd

F32 = mybir.dt.float32
F32R = mybir.dt.float32r
BF16 = mybir.dt.bfloat16
AF = mybir.ActivationFunctionType
ALU = mybir.AluOpType

NCORES = 8
B = 8192
Q = 4
D = 512
Bc = B // NCORES
CT = 512
NT = (Bc * Q) // CT
KC = D // 128

_cache = {}


def _build():
    nc = bacc.Bacc("TRN2", target_bir_lowering=False, debug=False,
                   num_devices=NCORES)

    def din(name, shape, dt=F32):
        return nc.dram_tensor(name, shape, dt, kind="ExternalInput")

    d_in = din("d", [Bc, 3])
    dth_in = din("dth", [3, Bc], F32R)
    dtl_in = din("dtl", [3, Bc], F32R)
    # split hidden weights, chunked [128, KC, D]
    a1h_in = din("a1h", [128, KC, D], F32R)
    a1l_in = din("a1l", [128, KC, D], F32R)
    a2h_in = din("a2h", [128, KC, D], F32R)
    a2l_in = din("a2l", [128, KC, D], F32R)
    a1th_in = din("a1th", [128, KC, D], F32R)
    a1tl_in = din("a1tl", [128, KC, D], F32R)
    b2h_in = din("b2h", [128, KC, D], F32R)
    b2l_in = din("b2l", [128, KC, D], F32R)
    # input weights [3, layer, D] hi/lo (lhsT for K=3 skip matmuls)
    wih_in = din("wih", [3, 3, D], F32R)
    wil_in = din("wil", [3, 3, D], F32R)
    # grad-stage lhsT [128, KC, layer, 3] hi/lo (layer2 slot holds C2T)
    wth_in = din("wth", [128, KC, 3, 3], F32R)
    wtl_in = din("wtl", [128, KC, 3, 3], F32R)
    nb_in = din("negb", [128, KC, 3 * Q])
    pb_in = din("posb", [128, KC, 3 * Q])
    out = nc.dram_tensor("out", [Bc, Q, 3], F32, kind="ExternalOutput")

    with tile.TileContext(nc) as tc:
        with (
            tc.tile_pool(name="wpool", bufs=1) as wp,
            tc.tile_pool(name="act", bufs=2) as ap,
            tc.tile_pool(name="ps", bufs=4, space="PSUM") as ps,
            tc.tile_pool(name="psg", bufs=2, space="PSUM") as psg,
            tc.tile_pool(name="fin", bufs=4) as fp,
            tc.tile_pool(name="dram", bufs=1, space="DRAM") as dp,
        ):
            def wload(src, dt=F32R, shape=None):
                t = wp.tile(shape or [128, KC, D], dt, tag=src.name)
                nc.sync.dma_start(t[:], src.ap())
                return t

            wih = wload(wih_in, shape=[3, 3, D])
            wil = wload(wil_in, shape=[3, 3, D])
            dth = wload(dth_in, shape=[3, Bc])
            dtl = wload(dtl_in, shape=[3, Bc])
            negb = wload(nb_in, F32, [128, KC, 3 * Q])
            posb = wload(pb_in, F32, [128, KC, 3 * Q])
            a1h, a1l = wload(a1h_in), wload(a1l_in)
            a2h, a2l = wload(a2h_in), wload(a2l_in)
            b2h, b2l = wload(b2h_in), wload(b2l_in)
            a1th, a1tl = wload(a1th_in), wload(a1tl_in)
            wth = wload(wth_in, shape=[128, KC, 3, 3])
            wtl = wload(wtl_in, shape=[128, KC, 3, 3])

            vdram = dp.tile([Bc, Q, 3], F32)

            def nbias(i, q, m):
                return negb[:, m, i * Q + q: i * Q + q + 1]

            def pbias(i, q, m):
                return posb[:, m, i * Q + q: i * Q + q + 1]

            # ---- final stage for one 128-direction chunk ----
            def final_chunk(ch):
                vt = fp.tile([128, Q * 3], F32, tag="vt")
                nc.sync.dma_start(vt[:], vdram[ch * 128:(ch + 1) * 128].rearrange("n q c -> n (q c)"))
                dt = fp.tile([128, 3], F32, tag="dt")
                nc.sync.dma_start(dt[:], d_in.ap()[ch * 128:(ch + 1) * 128])

                # dots_q = sum_c vt[:, 3q+c] * d[:, c]  (d cols as per-partition scalars)
                vq = vt[:].rearrange("p (q c) -> p c q", c=3)  # strided view
                tda = fp.tile([128, Q], F32, tag="tda")
                tdb = fp.tile([128, Q], F32, tag="tdb")
                dots = fp.tile([128, Q], F32, tag="dots")
                nc.vector.tensor_scalar_mul(tda[:], vq[:, 0], dt[:, 0:1])
                nc.vector.scalar_tensor_tensor(tdb[:], vq[:, 1], dt[:, 1:2],
                                               tda[:], ALU.mult, ALU.add)
                nc.vector.scalar_tensor_tensor(dots[:], vq[:, 2], dt[:, 2:3],
                                               tdb[:], ALU.mult, ALU.add)

                def dcol(qq):
                    return dots[:, qq:qq + 1]

                ra = fp.tile([128, Q], F32, tag="ra")
                rb = fp.tile([128, Q], F32, tag="rb")
                nc.vector.tensor_scalar(ra[:], dots[:], dcol(0), None, ALU.is_lt)
                for qq in range(1, Q):
                    src, dst = (ra, rb) if qq % 2 else (rb, ra)
                    nc.vector.scalar_tensor_tensor(dst[:], dots[:], dcol(qq),
                                                   src[:], ALU.is_lt, ALU.add)
                rk = rb
                tie = fp.tile([128, Q], F32, tag="tie")
                nc.vector.memset(tie[:, 0:1], 0.0)
                nc.vector.tensor_scalar(tie[:, 1:2], dcol(1), dcol(0), None, ALU.is_equal)
                t2 = fp.tile([128, 2], F32, tag="t2")
                nc.vector.tensor_scalar(t2[:, 0:1], dcol(2), dcol(0), None, ALU.is_equal)
                nc.vector.scalar_tensor_tensor(tie[:, 2:3], dcol(2), dcol(1),
                                               t2[:, 0:1], ALU.is_equal, ALU.add)
                t3 = fp.tile([128, 2], F32, tag="t3")
                nc.vector.tensor_scalar(t3[:, 0:1], dcol(3), dcol(0), None, ALU.is_equal)
                nc.vector.scalar_tensor_tensor(t3[:, 1:2], dcol(3), dcol(1),
                                               t3[:, 0:1], ALU.is_equal, ALU.add)
                nc.vector.scalar_tensor_tensor(tie[:, 3:4], dcol(3), dcol(2),
                                               t3[:, 1:2], ALU.is_equal, ALU.add)
                rkf = fp.tile([128, Q], F32, tag="rkf")
                nc.vector.tensor_tensor(rkf[:], rk[:], tie[:], ALU.add)

                ot = fp.tile([128, Q, 3], F32, tag="ot")
                sel = fp.tile([128, Q], F32, tag="sel")
                acc = fp.tile([128, 3], F32, tag="acc")
                for r in range(Q):
                    nc.vector.tensor_scalar(sel[:], rkf[:], float(r), None, ALU.is_equal)
                    nc.vector.tensor_scalar_mul(acc[:], vt[:, 0:3], sel[:, 0:1])
                    for qq in range(1, Q):
                        dst = ot[:, r] if qq == Q - 1 else acc
                        nc.vector.scalar_tensor_tensor(
                            dst, vt[:, 3 * qq:3 * qq + 3], sel[:, qq:qq + 1],
                            acc[:], ALU.mult, ALU.add)
                nc.sync.dma_start(
                    out.ap()[ch * 128:(ch + 1) * 128].rearrange("n q c -> n (q c)"),
                    ot[:].rearrange("p q c -> p (q c)"))


            vsb = [None, None, None]
            for t in range(NT):
                half, q = t // Q, t % Q
                cs = slice(half * CT, (half + 1) * CT)
                dh, dl = dth[:, cs], dtl[:, cs]

                if q == 0:
                    # V_i = d @ W_i for this direction-half (3 split terms)
                    for i in range(3):
                        vsb[i] = ap.tile([128, KC, CT], F32, tag=f"v{i}", bufs=1, name=f"v{i}_{half}")
                        for m in range(KC):
                            wslc = slice(m * 128, (m + 1) * 128)
                            pv = ps.tile([128, CT], F32, tag="pz")
                            nc.tensor.matmul(pv[:], wih[:, i, wslc], dh, start=True, stop=False)
                            nc.tensor.matmul(pv[:], wih[:, i, wslc], dl, start=False, stop=False)
                            nc.tensor.matmul(pv[:], wil[:, i, wslc], dh, start=False, stop=True)
                            nc.scalar.copy(vsb[i][:, m], pv[:])
                v0, v1, v2 = vsb

                def act_split(src_ap, i, m, hi_t, lo_t, mask_t, biased=False):
                    fc = ap.tile([128, CT], F32, tag="hfc", bufs=2)
                    if biased:
                        nc.scalar.activation(fc[:], src_ap, AF.Prelu,
                                             bias=0.0, scale=1.0, alpha=0.5)
                        nc.vector.tensor_scalar(mask_t[:, m], src_ap, 0.0, 0.5,
                                                ALU.is_ge, ALU.max)
                    else:
                        nc.scalar.activation(fc[:], src_ap, AF.Prelu,
                                             bias=pbias(i, q, m), scale=1.0, alpha=0.5)
                        nc.vector.tensor_scalar(mask_t[:, m], src_ap, nbias(i, q, m), 0.5,
                                                ALU.is_ge, ALU.max)
                    nc.vector.tensor_copy(hi_t[:, m], fc[:])
                    nc.vector.tensor_sub(lo_t[:, m], fc[:], hi_t[:, m])

                # ---- L0 ----
                h0h = ap.tile([128, KC, CT], F32R, tag="h0h", bufs=1)
                h0l = ap.tile([128, KC, CT], F32R, tag="h0l", bufs=1)
                m0 = ap.tile([128, KC, CT], BF16, tag="m0", bufs=1)
                for m in range(KC):
                    act_split(v0[:, m], 0, m, h0h, h0l, m0)

                # ---- L1 ----
                h1h = ap.tile([128, KC, CT], F32R, tag="h1h", bufs=1)
                h1l = ap.tile([128, KC, CT], F32R, tag="h1l", bufs=1)
                m1 = ap.tile([128, KC, CT], BF16, tag="m1", bufs=1)
                for m in range(KC):
                    wslc = slice(m * 128, (m + 1) * 128)
                    pz = ps.tile([128, CT], F32, tag="pz")
                    for k in range(KC):
                        nc.tensor.matmul(pz[:], a1h[:, k, wslc], h0h[:, k], start=(k == 0), stop=False)
                        nc.tensor.matmul(pz[:], a1h[:, k, wslc], h0l[:, k], start=False, stop=False)
                        nc.tensor.matmul(pz[:], a1l[:, k, wslc], h0h[:, k],
                                         start=False, stop=(k == KC - 1))
                    zf = ap.tile([128, CT], F32, tag="zf", bufs=2)
                    nc.vector.scalar_tensor_tensor(zf[:], pz[:], pbias(1, q, m),
                                                   v1[:, m], ALU.add, ALU.add)
                    act_split(zf[:], 1, m, h1h, h1l, m1, biased=True)

                # ---- L2 ----
                m2 = ap.tile([128, KC, CT], F32R, tag="m2", bufs=1)
                for m in range(KC):
                    wslc = slice(m * 128, (m + 1) * 128)
                    pz = ps.tile([128, CT], F32, tag="pz")
                    for k in range(KC):
                        nc.tensor.matmul(pz[:], a2h[:, k, wslc], h1h[:, k], start=(k == 0), stop=False)
                        nc.tensor.matmul(pz[:], a2h[:, k, wslc], h1l[:, k], start=False, stop=False)
                        nc.tensor.matmul(pz[:], a2l[:, k, wslc], h1h[:, k],
                                         start=False, stop=(k == KC - 1))
                    zf = ap.tile([128, CT], F32, tag="zf", bufs=2)
                    nc.vector.scalar_tensor_tensor(zf[:], pz[:], pbias(2, q, m),
                                                   v2[:, m], ALU.add, ALU.add)
                    nc.vector.tensor_scalar(m2[:, m], zf[:], 0.0, 0.5,
                                            ALU.is_ge, ALU.max)

                # ---- B2: g1 = (m2 @ B2) * m1 ----
                g1h = ap.tile([128, KC, CT], F32R, tag="g1h", bufs=1)
                g1l = ap.tile([128, KC, CT], F32R, tag="g1l", bufs=1)
                for m in range(KC):
                    wslc = slice(m * 128, (m + 1) * 128)
                    pz = ps.tile([128, CT], F32, tag="pz")
                    for k in range(KC):
                        nc.tensor.matmul(pz[:], b2h[:, k, wslc], m2[:, k],
                                         start=(k == 0), stop=False)
                        nc.tensor.matmul(pz[:], b2l[:, k, wslc], m2[:, k],
                                         start=False, stop=(k == KC - 1))
                    gc = ap.tile([128, CT], F32, tag="hfc", bufs=2)
                    nc.vector.tensor_tensor(gc[:], pz[:], m1[:, m], ALU.mult)
                    nc.vector.tensor_copy(g1h[:, m], gc[:])
                    nc.vector.tensor_sub(g1l[:, m], gc[:], g1h[:, m])

                # ---- B1: g0 = (g1 @ A1^T) * m0 ----
                g0h = ap.tile([128, KC, CT], F32R, tag="g0h", bufs=1)
                g0l = ap.tile([128, KC, CT], F32R, tag="g0l", bufs=1)
                for m in range(KC):
                    wslc = slice(m * 128, (m + 1) * 128)
                    pz = ps.tile([128, CT], F32, tag="pz")
                    for k in range(KC):
                        nc.tensor.matmul(pz[:], a1th[:, k, wslc], g1h[:, k], start=(k == 0), stop=False)
                        nc.tensor.matmul(pz[:], a1th[:, k, wslc], g1l[:, k], start=False, stop=False)
                        nc.tensor.matmul(pz[:], a1tl[:, k, wslc], g1h[:, k],
                                         start=False, stop=(k == KC - 1))
                    gc = ap.tile([128, CT], F32, tag="hfc", bufs=2)
                    nc.vector.tensor_tensor(gc[:], pz[:], m0[:, m], ALU.mult)
                    nc.vector.tensor_copy(g0h[:, m], gc[:])
                    nc.vector.tensor_sub(g0l[:, m], gc[:], g0h[:, m])

                # ---- grad^T [3, CT] ----
                pg = psg.tile([3, CT], F32, tag="pg")
                first = True
                for k in range(KC):
                    nc.tensor.matmul(pg[:], wth[:, k, 2], m2[:, k], start=first, stop=False)
                    first = False
                    nc.tensor.matmul(pg[:], wtl[:, k, 2], m2[:, k], start=False, stop=False)
                for k in range(KC):
                    nc.tensor.matmul(pg[:], wth[:, k, 1], g1h[:, k], start=False, stop=False)
                    nc.tensor.matmul(pg[:], wth[:, k, 1], g1l[:, k], start=False, stop=False)
                    nc.tensor.matmul(pg[:], wtl[:, k, 1], g1h[:, k], start=False, stop=False)
                for k in range(KC):
                    nc.tensor.matmul(pg[:], wth[:, k, 0], g0h[:, k], start=False, stop=False)
                    nc.tensor.matmul(pg[:], wth[:, k, 0], g0l[:, k], start=False, stop=False)
                    nc.tensor.matmul(pg[:], wtl[:, k, 0], g0h[:, k], start=False,
                                     stop=(k == KC - 1))
                gsb = fp.tile([3, CT], F32, tag="gsb", bufs=2)
                nc.scalar.copy(gsb[:], pg[:])
                nc.sync.dma_start(
                    vdram[half * CT:(half + 1) * CT, q, :].rearrange("n c -> c n"),
                    gsb[:])

                if q == Q - 1:
                    for ch in range(half * 4, half * 4 + 4):
                        final_chunk(ch)

    nc.compile()
    return nc


def _r11(x):
    b = np.ascontiguousarray(x, np.float32).view(np.uint32)
    return ((b + np.uint32(1 << 11)) & np.uint32(0xFFFFF000)).view(np.float32)


def _split(x):
    hi = _r11(x)
    lo = _r11((x - hi).astype(np.float32))
    return hi, lo


def _chunk(M):  # [D, X...] -> [128, KC, X...]
    return np.ascontiguousarray(M.reshape(KC, 128, *M.shape[1:]).transpose(
        1, 0, *range(2, M.ndim + 1)))


def _prep(directions, perturbations, W_in, W_hid, w_out):
    A1 = np.abs(np.asarray(W_hid[0], np.float64))
    A2 = np.abs(np.asarray(W_hid[1], np.float64))
    w = np.abs(np.asarray(w_out, np.float64))
    W = np.asarray(W_in, np.float64)           # [3, 3, D]
    p = np.asarray(perturbations, np.float64)

    A1f, A2f = A1.astype(np.float32), A2.astype(np.float32)
    B2 = (w[:, None] * A2.T).astype(np.float32)
    C2T = (w[:, None] * W[2].T).astype(np.float32)   # [D, 3]
    bias = np.einsum("icd,qc->iqd", W, p).astype(np.float32)

    def sp(M):
        return _split(np.asarray(M, np.float32))

    a1h, a1l = sp(A1f)
    a2h, a2l = sp(A2f)
    a1th, a1tl = sp(np.ascontiguousarray(A1f.T))
    b2h, b2l = sp(B2)
    wf = W.astype(np.float32)                 # [3(layer), 3(c), D]
    wih, wil = sp(np.transpose(wf, (1, 0, 2)))  # [3(c), layer, D]
    # grad lhsT per layer: [D, 3]; layer2 -> C2T
    wtall = np.stack([np.ascontiguousarray(wf[0].T),
                      np.ascontiguousarray(wf[1].T),
                      C2T], axis=1)            # [D, layer, 3]
    wth, wtl = sp(wtall)

    common = {
        "a1h": _chunk(a1h), "a1l": _chunk(a1l),
        "a2h": _chunk(a2h), "a2l": _chunk(a2l),
        "a1th": _chunk(a1th), "a1tl": _chunk(a1tl),
        "b2h": _chunk(b2h), "b2l": _chunk(b2l),
        "wih": np.ascontiguousarray(wih), "wil": np.ascontiguousarray(wil),
        "wth": _chunk(wth), "wtl": _chunk(wtl),
        "negb": _chunk(np.transpose(-bias, (2, 0, 1)).reshape(D, 3 * Q)),
        "posb": _chunk(np.transpose(bias, (2, 0, 1)).reshape(D, 3 * Q)),
    }
    d = np.asarray(directions, np.float32)
    in_maps = []
    for c in range(NCORES):
        ds = np.ascontiguousarray(d[c * Bc:(c + 1) * Bc])
        dT = np.ascontiguousarray(ds.T)
        dth, dtl = _split(dT)
        m = dict(common)
        m["d"] = ds
        m["dth"] = dth
        m["dtl"] = dtl
        in_maps.append(m)
    return in_maps


def kernel(directions, perturbations, W_in, W_hid, w_out):
    if "nc" not in _cache:
        _cache["nc"] = _build()
    nc = _cache["nc"]
    in_maps = _prep(directions, perturbations, W_in, W_hid, w_out)
    res = run_bass_kernel_spmd(nc, in_maps, core_ids=list(range(NCORES)))
    outp = np.concatenate([r["out"] for r in res.results], axis=0)
    return outp.astype(np.float32)
